# revision 29
# baseline (speedup 1.0000x reference)
"""Multi-head causal self-attention with RoPE on 8 Trainium2 NeuronCores.

Sharding: 16 heads -> 8 cores (2 heads/core, head/tensor parallel).
Wq/Wk/Wv column-sharded (per-head-group rows of W), Wo row-sharded.
Each core computes a full (S, D) partial of the output projection in fp16;
the host sums the 8 partials (the row-parallel reduce).

v2 layout notes (cost-model driven):
 - everything fp16 on SBUF; psum fp32; moving matmul operands fp16 so every
   matmul runs at 1 cycle/column regardless of width.
 - attnV computed in [q_part, dk_free] layout: per 128-query subchunk an
   accumulating psum [128, 2*66] ([v|Z] per head), with es chunks as the
   stationary operand -> 65-column matmuls instead of 512-column ones.
 - softmax normalization: per-partition reciprocal of the Z column + one
   tensor_scalar multiply per head -> fp16 attn tile, then a DMA-transpose
   (14ns/tile) back to [dk, q] for the output projection.
 - exp work split: Activation engine does most chunks (exact exp); ~1/3 of
   the off-diagonal chunks run on DVE via a Schraudolph int16 bit-trick
   (out_i16 = round(s*A + B) bitcast fp16 ~= exp(s/8), rel err ~2%).
 - causal diagonal chunks compute only the valid score columns (narrow
   matmuls + narrow per-head exp) and are masked on GPSIMD.
 - output projection psums drain via DVE/Act copies into a per-qtile fp16
   row buffer, one DMA per qtile to DRAM.
"""

import sys

for _p in ("/opt/trn_rl_repo", "/root/.axon_site/_ro/trn_rl_repo"):
    if _p not in sys.path:
        sys.path.insert(0, _p)

import numpy as np

S_FULL = 4096
D = 1024
NH = 16
DK = 64
P = 128
QT = 512  # q tile (free dim of score tiles)
KC = 128  # k chunk (partition dim of score tiles)
DC = D // P  # 8 contraction chunks for the projections
THETA = 10000.0
N_CORES = 8

# Schraudolph exp constants for fp16 bit pattern: round(s*A + B) ~ fp16(exp(s/8))
LOG2E = 1.4426950408889634
EXP_A = (1 << 10) * LOG2E * 0.125
EXP_B = 15.0 * (1 << 10) - 58.9

_BUILD_CACHE: dict = {}


def _dve_exp_set(S: int) -> set:
    """(qt, kc) pairs whose exp runs on DVE via the bit trick (off-diagonal
    chunks only; roughly 1/4 of them, interleaved)."""
    NQ = S // QT
    DIAG = QT // KC
    out = set()
    idx = 0
    for qt in range(NQ):
        for kc in range(DIAG * qt):  # off-diagonal only
            if idx % 16 < 4:
                out.add((qt, kc))
            idx += 1
    return out


def build(S: int = S_FULL, reps: int = 1):
    """Build the per-core Bass program (same program for all cores)."""
    key = (S, reps)
    if key in _BUILD_CACHE:
        return _BUILD_CACHE[key]

    import concourse.bacc as bacc
    import concourse.tile as tile
    from concourse import mybir

    f32 = mybir.dt.float32
    f16 = mybir.dt.float16
    i16 = mybir.dt.int16
    Alu = mybir.AluOpType
    Act = mybir.ActivationFunctionType

    NQ = S // QT
    NK = S // KC
    DIAG = QT // KC  # k-chunks per q-tile on the diagonal (4)
    dve_exp = _dve_exp_set(S)
    SCALE = float(DK) ** -0.5

    nc = bacc.Bacc(
        "TRN2", target_bir_lowering=False, debug=False, num_devices=N_CORES
    )
    xT = nc.dram_tensor("xT", [D, S], f16, kind="ExternalInput")
    wqT = nc.dram_tensor("wqT", [D, P], f16, kind="ExternalInput")
    wkT = nc.dram_tensor("wkT", [D, P], f16, kind="ExternalInput")
    wvT = nc.dram_tensor("wvT", [D, P], f16, kind="ExternalInput")
    woT = nc.dram_tensor("woT", [P, D], f16, kind="ExternalInput")
    cosd = nc.dram_tensor("cosd", [P, S], f16, kind="ExternalInput")
    sind = nc.dram_tensor("sind", [P, S], f16, kind="ExternalInput")
    maskd = nc.dram_tensor("maskd", [P, DIAG, 2, QT], f16, kind="ExternalInput")
    ident = nc.dram_tensor("ident", [P, P], f16, kind="ExternalInput")
    onesv = nc.dram_tensor("onesv", [P, NK, 1], f16, kind="ExternalInput")
    yT = nc.dram_tensor("yT", [D, S], f16, kind="ExternalOutput")

    with tile.TileContext(nc) as tc:
        with (
            tc.tile_pool(name="const", bufs=1) as cp,
            tc.tile_pool(name="persist", bufs=1) as pp,
        ):
            # ---- constants ----
            wq_sb = cp.tile([P, DC, P], f16, tag="wq")
            wk_sb = cp.tile([P, DC, P], f16, tag="wk")
            wv_sb = cp.tile([P, DC, P], f16, tag="wv")
            wo_sb = cp.tile([P, D], f16, tag="wo")
            mask_sb = cp.tile([P, DIAG, 2, QT], f16, tag="mask")
            id_sb = cp.tile([P, P], f16, tag="ident")

            nc.sync.dma_start(out=wq_sb, in_=wqT[:, :].rearrange("(c p) m -> p c m", p=P))
            nc.sync.dma_start(out=wk_sb, in_=wkT[:, :].rearrange("(c p) m -> p c m", p=P))
            nc.sync.dma_start(out=wv_sb, in_=wvT[:, :].rearrange("(c p) m -> p c m", p=P))
            nc.sync.dma_start(out=id_sb, in_=ident[:, :])

            # ---- persistent activations ----
            qT_sb = pp.tile([P, S], f16, tag="qT")
            kT_sb = pp.tile([P, S], f16, tag="kT")
            v1a = pp.tile([P, NK, 65], f16, tag="v1a")  # head 0: [v, ones]
            v1b = pp.tile([P, NK, 65], f16, tag="v1b")  # head 1
            # es double-buffered by qt parity so exp(qt) can overwrite while
            # attnV(qt-1) still reads the other buffer
            es_ev = pp.tile([P, NK - DIAG, 2, QT], f16, tag="esev")
            es_od = pp.tile([P, NK, 2, QT], f16, tag="esod")
            attnT = pp.tile([P, 2, QT], f16, tag="attnT")  # parity-buffered

            # ---- phase A: projections + RoPE + v-transposes, per 512-col chunk ----
            with (
                tc.tile_pool(name="xc", bufs=3) as xcp,
                tc.tile_pool(name="rope", bufs=2) as rp,
                tc.tile_pool(name="proj_ps", bufs=2, space="PSUM") as pps,
                tc.tile_pool(name="tp_ps", bufs=2, space="PSUM") as tpp,
            ):
                # prefetch chunk DMAs ahead of the bulky const loads
                def fetch_chunk(nt):
                    sl = slice(nt * QT, (nt + 1) * QT)
                    xc = xcp.tile([P, DC, QT], f16, tag="xc")
                    xv = xT[:, sl].rearrange("(c p) q -> p c q", p=P)
                    if nt == 0:
                        # split so the first projection matmuls start sooner
                        nc.sync.dma_start(out=xc[:, 0:2, :], in_=xv[:, 0:2, :])
                        nc.sync.dma_start(out=xc[:, 2:DC, :], in_=xv[:, 2:DC, :])
                    else:
                        nc.sync.dma_start(out=xc, in_=xv)
                    cos_c = rp.tile([P, QT], f16, tag="cosc")
                    sin_c = rp.tile([P, QT], f16, tag="sinc")
                    nc.sync.dma_start(out=cos_c, in_=cosd[:, sl])
                    nc.sync.dma_start(out=sin_c, in_=sind[:, sl])
                    return xc, cos_c, sin_c

                pref = fetch_chunk(0)
                # phase-E-only constants load behind the first x chunk
                nc.sync.dma_start(out=wo_sb, in_=woT[:, :])
                nc.sync.dma_start(out=mask_sb, in_=maskd[:, :, :, :])
                nc.sync.dma_start(out=v1a[:, :, 64:65], in_=onesv[:, :, :])
                nc.sync.dma_start(out=v1b[:, :, 64:65], in_=onesv[:, :, :])

                vt_prev = None
                for nt in range(NQ):
                    sl = slice(nt * QT, (nt + 1) * QT)
                    xc, cos_c, sin_c = pref
                    if nt + 1 < NQ:
                        pref = fetch_chunk(nt + 1)
                    psq = pps.tile([P, QT], f32, tag="psq")
                    psk = pps.tile([P, QT], f32, tag="psk")
                    psv = pps.tile([P, QT], f32, tag="psv")
                    for c in range(DC):
                        st, sp = (c == 0), (c == DC - 1)
                        nc.tensor.matmul(psq, wq_sb[:, c, :], xc[:, c, :], start=st, stop=sp)
                        nc.tensor.matmul(psk, wk_sb[:, c, :], xc[:, c, :], start=st, stop=sp)
                        nc.tensor.matmul(psv, wv_sb[:, c, :], xc[:, c, :], start=st, stop=sp)
                    vt = rp.tile([P, QT], f16, tag="vt")
                    nc.scalar.copy(qT_sb[:, sl], psq)
                    nc.scalar.copy(kT_sb[:, sl], psk)
                    nc.scalar.copy(vt, psv)
                    # RoPE on this chunk (in place); sw-mul runs on GPSIMD;
                    # q swap DMAs issue from SP, k swaps from Act (spread SEQ load)
                    for src_sb, tgname, dmaeng in (
                        (qT_sb, "swq", nc.sync),
                        (kT_sb, "swk", nc.scalar),
                    ):
                        sw = rp.tile([P, QT], f16, tag=tgname)
                        for dst0, src0 in ((0, 32), (32, 0), (64, 96), (96, 64)):
                            dmaeng.dma_start(
                                out=sw[dst0 : dst0 + 32, :],
                                in_=src_sb[src0 : src0 + 32, sl],
                            )
                        m1 = rp.tile([P, QT], f16, tag=tgname + "m")
                        nc.vector.tensor_mul(m1, src_sb[:, sl], cos_c)
                        nc.gpsimd.tensor_mul(sw, sw, sin_c)
                        nc.vector.tensor_add(src_sb[:, sl], m1, sw)
                    # v~ transposes lag one chunk so PE never waits on the
                    # fresh vt copy
                    def vtrans(nt, vt):
                        for h, v1 in ((0, v1a), (1, v1b)):
                            hp = h * 64
                            pst = tpp.tile([P, DIAG, 64], f16, tag="pst", name="pst")
                            for j in range(DIAG):
                                nc.tensor.transpose(
                                    pst[:, j, :],
                                    vt[hp : hp + 64, j * KC : (j + 1) * KC],
                                    id_sb[hp : hp + 64, hp : hp + 64],
                                )
                            nc.vector.tensor_copy(
                                v1[:, DIAG * nt : DIAG * nt + DIAG, 0:64], pst
                            )
                    if vt_prev is not None:
                        vtrans(*vt_prev)
                    vt_prev = (nt, vt)
                if vt_prev is not None:
                    vtrans(*vt_prev)

            # ---- phase E: scores+exp, attnV, outproj, per q tile ----
            with (
                tc.tile_pool(name="sc_ps", bufs=3, space="PSUM") as scp,
                tc.tile_pool(name="att_ps", bufs=1, space="PSUM") as attp,
                tc.tile_pool(name="po_ps", bufs=1, space="PSUM") as pop,
                tc.tile_pool(name="asb", bufs=4) as asp,
                tc.tile_pool(name="yrow", bufs=1) as yrp,
            ):
                def score_unit(qt, kc):
                    q0 = qt * QT
                    es_all = es_od if qt % 2 else es_ev
                    if True:
                        j = kc - DIAG * qt  # >=0 on the diagonal
                        qoff = j * KC if j >= 0 else 0
                        ksl = slice(kc * KC, (kc + 1) * KC)
                        ps = scp.tile([P, 2, QT], f32, tag="sc")
                        nc.tensor.matmul(
                            ps[:, 0, qoff:QT],
                            kT_sb[0:64, ksl],
                            qT_sb[0:64, q0 + qoff : q0 + QT],
                            start=True, stop=True, tile_position=(0, 0),
                        )
                        nc.tensor.matmul(
                            ps[:, 1, qoff:QT],
                            kT_sb[64:128, ksl],
                            qT_sb[64:128, q0 + qoff : q0 + QT],
                            start=True, stop=True, tile_position=(64, 0),
                        )
                        if j >= 2:
                            # narrow per-head DVE trick exp (masked after)
                            for h in range(2):
                                nc.vector.tensor_scalar(
                                    out=es_all.bitcast(i16)[:, kc, h, qoff:QT],
                                    in0=ps[:, h, qoff:QT],
                                    scalar1=float(EXP_A),
                                    scalar2=float(EXP_B),
                                    op0=Alu.mult,
                                    op1=Alu.add,
                                )
                        elif j >= 0:
                            # narrow per-head exact exp
                            for h in range(2):
                                nc.scalar.activation(
                                    es_all[:, kc, h, qoff:QT],
                                    ps[:, h, qoff:QT],
                                    Act.Exp,
                                    scale=SCALE,
                                )
                        elif (qt, kc) in dve_exp:
                            nc.vector.tensor_scalar(
                                out=es_all.bitcast(i16)[:, kc, :, :].rearrange(
                                    "p h q -> p (h q)"
                                ),
                                in0=ps.rearrange("p h q -> p (h q)"),
                                scalar1=float(EXP_A),
                                scalar2=float(EXP_B),
                                op0=Alu.mult,
                                op1=Alu.add,
                            )
                        else:
                            nc.scalar.activation(
                                es_all[:, kc, :, :].rearrange("p h q -> p (h q)"),
                                ps.rearrange("p h q -> p (h q)"),
                                Act.Exp,
                                scale=SCALE,
                            )
                        if j >= 0:
                            nc.gpsimd.tensor_mul(
                                es_all[:, kc, :, qoff:QT],
                                es_all[:, kc, :, qoff:QT],
                                mask_sb[:, j, :, qoff:QT],
                            )
                def attn_units(qt):
                    """Closures for attnV matmul steps, norms, and outproj of
                    q-tile qt; interleaved between scores chunks of qt+1 so
                    the in-order PE queue always has ready work when scores
                    stall on psum banks."""
                    es_all = es_od if qt % 2 else es_ev
                    par = qt % 2
                    q0 = qt * QT
                    state = {}
                    units = []

                    def mk_av(qcl, kc, qc):
                        def f():
                            if kc == 0:
                                state[qcl] = attp.tile([P, 2, 66], f32, tag="att", name="pa")
                            pa = state[qcl]
                            st, sp = (kc == 0), (kc == qc)
                            nc.tensor.matmul(
                                pa[:, 0, 0:65],
                                es_all[:, kc, 0, qcl * KC : (qcl + 1) * KC],
                                v1a[:, kc, :],
                                start=st, stop=sp,
                            )
                            nc.tensor.matmul(
                                pa[:, 1, 0:65],
                                es_all[:, kc, 1, qcl * KC : (qcl + 1) * KC],
                                v1b[:, kc, :],
                                start=st, stop=sp,
                            )
                        return f

                    def mk_norm(qcl):
                        def f():
                            pa = state[qcl]
                            nc.vector.reciprocal(pa[:, :, 64:65], pa[:, :, 64:65])
                            asb = asp.tile([P, P], f16, tag="asb", name="asb")
                            for h in range(2):
                                nc.vector.tensor_scalar(
                                    out=asb[:, h * 64 : (h + 1) * 64],
                                    in0=pa[:, h, 0:64],
                                    scalar1=pa[:, h, 64:65],
                                    scalar2=None,
                                    op0=Alu.mult,
                                )
                            nc.sync.dma_start_transpose(
                                out=attnT[:, par, qcl * KC : (qcl + 1) * KC],
                                in_=asb,
                            )
                        return f

                    def mk_po(oc):
                        def f():
                            if oc == 0:
                                state["yrow"] = yrp.tile([P, DC, QT], f16, tag="yrow", name="yrow")
                            po = pop.tile([P, QT], f32, tag="po", name="po")
                            nc.tensor.matmul(
                                po, wo_sb[:, oc * P : (oc + 1) * P],
                                attnT[:, par, :],
                                start=True, stop=True,
                            )
                            # tail iterations have little interleave cover:
                            # alternate the psum drain across Act/DVE there
                            if qt >= NQ - 2 and oc % 2:
                                nc.scalar.copy(state["yrow"][:, oc, :], po)
                            else:
                                nc.vector.tensor_copy(state["yrow"][:, oc, :], po)
                            if oc == DC - 1:
                                qsl = slice(q0, q0 + QT)
                                nc.scalar.dma_start(
                                    out=yT[:, qsl].rearrange("(c p) q -> p c q", p=P),
                                    in_=state["yrow"],
                                )
                        return f

                    av_units = []
                    po_units = []
                    for qcl in range(DIAG):
                        qc = DIAG * qt + qcl
                        for kc in range(qc + 1):
                            av_units.append(mk_av(qcl, kc, qc))
                        av_units.append(mk_norm(qcl))
                    for oc in range(DC):
                        po_units.append(mk_po(oc))
                    return av_units, po_units

                # software pipeline: attnV/norm of qt-1 and outproj of qt-2
                # interleave between the scores chunks of qt (proportional
                # merge) so the in-order PE queue always has ready work
                avpo = {}
                for it in range(NQ + 2):
                    units = []
                    if 1 <= it <= NQ:
                        avpo[it - 1] = attn_units(it - 1)
                        units += avpo[it - 1][0]
                    if it >= 2:
                        units += avpo.pop(it - 2)[1]
                    if it < NQ:
                        nkc = DIAG * it + DIAG
                        done = 0
                        for kc in range(nkc):
                            score_unit(it, kc)
                            want = (kc + 1) * len(units) // nkc
                            while done < want:
                                units[done]()
                                done += 1
                    else:
                        # tail: interleave attnV and outproj streams
                        a, b = (avpo[it - 1][0], units[len(avpo[it - 1][0]):]) if (
                            it - 1
                        ) in avpo else (units, [])
                        ia = ib = 0
                        n = max(len(a), 1)
                        for i_ in range(len(a)):
                            a[i_]()
                            want = (i_ + 1) * len(b) // n
                            while ib < want:
                                b[ib]()
                                ib += 1
                        while ib < len(b):
                            b[ib]()
                            ib += 1

    nc.compile()
    _BUILD_CACHE[key] = nc
    return nc


def host_prep(x, Wq, Wk, Wv, Wo, S=S_FULL):
    """Build per-core input maps (numpy, fp16)."""
    x = np.asarray(x, np.float32).reshape(S, D)
    xT = np.ascontiguousarray(x.T, dtype=np.float16)

    perm64 = np.concatenate([np.arange(0, 64, 2), np.arange(1, 64, 2)])
    j32 = np.arange(32, dtype=np.float64)
    rates = THETA ** (-2.0 * j32 / DK)
    pos = np.arange(S, dtype=np.float64)
    ang = rates[:, None] * pos[None, :]  # (32, S)
    cos32 = np.cos(ang)
    sin32 = np.sin(ang)
    cosd = np.tile(cos32, (4, 1)).astype(np.float16)  # (128, S)
    signs = np.repeat([-1.0, 1.0, -1.0, 1.0], 32)[:, None]
    sind = (np.tile(sin32, (4, 1)) * signs).astype(np.float16)

    DIAG = QT // KC
    r = np.arange(P)[:, None, None, None]
    jj = np.arange(DIAG)[None, :, None, None]
    q_local = np.arange(QT)[None, None, None, :]
    maskd = np.broadcast_to(
        (q_local >= jj * KC + r), (P, DIAG, 2, QT)
    ).astype(np.float16)

    ident = np.eye(P, dtype=np.float16)

    in_maps = []
    for g in range(N_CORES):
        h0, h1 = 2 * g, 2 * g + 1
        idx_qk = np.concatenate([h0 * DK + perm64, h1 * DK + perm64])
        idx_v = np.arange(h0 * DK, h0 * DK + 2 * DK)
        in_maps.append(
            {
                "xT": xT,
                "wqT": np.ascontiguousarray(np.asarray(Wq)[idx_qk, :].T, dtype=np.float16),
                "wkT": np.ascontiguousarray(np.asarray(Wk)[idx_qk, :].T, dtype=np.float16),
                "wvT": np.ascontiguousarray(np.asarray(Wv)[idx_v, :].T, dtype=np.float16),
                "woT": np.ascontiguousarray(np.asarray(Wo)[:, idx_v].T, dtype=np.float16),
                "cosd": cosd,
                "sind": sind,
                "maskd": maskd,
                "ident": ident,
                "onesv": np.ones((P, S // KC, 1), np.float16),
            }
        )
    return in_maps


def run_cores(x, Wq, Wk, Wv, Wo, S=S_FULL, core_ids=None, trace=False):
    from concourse.bass_utils import run_bass_kernel_spmd

    nc = build(S)
    in_maps = host_prep(x, Wq, Wk, Wv, Wo, S=S)
    if core_ids is None:
        core_ids = list(range(N_CORES))
    in_maps = in_maps[: len(core_ids)]
    res = run_bass_kernel_spmd(nc, in_maps, core_ids, trace=trace)
    return res


def kernel(x, Wq, Wk, Wv, Wo):
    x = np.asarray(x, np.float32)
    res = run_cores(x, np.asarray(Wq), np.asarray(Wk), np.asarray(Wv), np.asarray(Wo))
    y = np.zeros((D, S_FULL), np.float64)
    for r in res.results:
        y += r["yT"].astype(np.float64)
    return np.ascontiguousarray(y.T, dtype=np.float32).reshape(1, S_FULL, D)


# revision 30
# speedup vs baseline: 1.0381x; 1.0381x over previous
"""Multi-head causal self-attention with RoPE on 8 Trainium2 NeuronCores.

Sharding: 16 heads -> 8 cores (2 heads/core, head/tensor parallel).
Wq/Wk/Wv column-sharded (per-head-group rows of W), Wo row-sharded.
Each core computes a full (S, D) partial of the output projection in fp16;
the host sums the 8 partials (the row-parallel reduce).

v2 layout notes (cost-model driven):
 - everything fp16 on SBUF; psum fp32; moving matmul operands fp16 so every
   matmul runs at 1 cycle/column regardless of width.
 - attnV computed in [q_part, dk_free] layout: per 128-query subchunk an
   accumulating psum [128, 2*66] ([v|Z] per head), with es chunks as the
   stationary operand -> 65-column matmuls instead of 512-column ones.
 - softmax normalization: per-partition reciprocal of the Z column + one
   tensor_scalar multiply per head -> fp16 attn tile, then a DMA-transpose
   (14ns/tile) back to [dk, q] for the output projection.
 - exp work split: Activation engine does most chunks (exact exp); ~1/3 of
   the off-diagonal chunks run on DVE via a Schraudolph int16 bit-trick
   (out_i16 = round(s*A + B) bitcast fp16 ~= exp(s/8), rel err ~2%).
 - causal diagonal chunks compute only the valid score columns (narrow
   matmuls + narrow per-head exp) and are masked on GPSIMD.
 - output projection psums drain via DVE/Act copies into a per-qtile fp16
   row buffer, one DMA per qtile to DRAM.
"""

import sys

for _p in ("/opt/trn_rl_repo", "/root/.axon_site/_ro/trn_rl_repo"):
    if _p not in sys.path:
        sys.path.insert(0, _p)

import numpy as np

S_FULL = 4096
D = 1024
NH = 16
DK = 64
P = 128
QT = 512  # q tile (free dim of score tiles)
KC = 128  # k chunk (partition dim of score tiles)
DC = D // P  # 8 contraction chunks for the projections
THETA = 10000.0
N_CORES = 8

# Schraudolph exp constants for fp16 bit pattern: round(s*A + B) ~ fp16(exp(s/8))
LOG2E = 1.4426950408889634
EXP_A = (1 << 10) * LOG2E * 0.125
EXP_B = 15.0 * (1 << 10) - 58.9

_BUILD_CACHE: dict = {}


def _dve_exp_set(S: int) -> set:
    """(qt, kc) pairs whose exp runs on DVE via the bit trick (off-diagonal
    chunks only; roughly 1/4 of them, interleaved)."""
    NQ = S // QT
    DIAG = QT // KC
    out = set()
    idx = 0
    for qt in range(NQ):
        for kc in range(DIAG * qt):  # off-diagonal only
            if idx % 16 < 4:
                out.add((qt, kc))
            idx += 1
    return out


def build(S: int = S_FULL, reps: int = 1):
    """Build the per-core Bass program (same program for all cores)."""
    key = (S, reps)
    if key in _BUILD_CACHE:
        return _BUILD_CACHE[key]

    import concourse.bacc as bacc
    import concourse.tile as tile
    from concourse import mybir

    f32 = mybir.dt.float32
    f16 = mybir.dt.float16
    i16 = mybir.dt.int16
    Alu = mybir.AluOpType
    Act = mybir.ActivationFunctionType

    NQ = S // QT
    NK = S // KC
    DIAG = QT // KC  # k-chunks per q-tile on the diagonal (4)
    dve_exp = _dve_exp_set(S)
    SCALE = float(DK) ** -0.5

    nc = bacc.Bacc(
        "TRN2", target_bir_lowering=False, debug=False, num_devices=N_CORES
    )
    xT = nc.dram_tensor("xT", [D, S], f16, kind="ExternalInput")
    wqT = nc.dram_tensor("wqT", [D, P], f16, kind="ExternalInput")
    wkT = nc.dram_tensor("wkT", [D, P], f16, kind="ExternalInput")
    wvT = nc.dram_tensor("wvT", [D, P], f16, kind="ExternalInput")
    woT = nc.dram_tensor("woT", [P, D], f16, kind="ExternalInput")
    cosd = nc.dram_tensor("cosd", [P, S], f16, kind="ExternalInput")
    sind = nc.dram_tensor("sind", [P, S], f16, kind="ExternalInput")
    maskd = nc.dram_tensor("maskd", [P, DIAG, 2, QT], f16, kind="ExternalInput")
    ident = nc.dram_tensor("ident", [P, P], f16, kind="ExternalInput")
    onesv = nc.dram_tensor("onesv", [P, NK, 1], f16, kind="ExternalInput")
    yT = nc.dram_tensor("yT", [D, S], f16, kind="ExternalOutput")

    with tile.TileContext(nc) as tc:
        with (
            tc.tile_pool(name="const", bufs=1) as cp,
            tc.tile_pool(name="persist", bufs=1) as pp,
        ):
            # ---- constants ----
            wq_sb = cp.tile([P, DC, P], f16, tag="wq")
            wk_sb = cp.tile([P, DC, P], f16, tag="wk")
            wv_sb = cp.tile([P, DC, P], f16, tag="wv")
            wo_sb = cp.tile([P, D], f16, tag="wo")
            mask_sb = cp.tile([P, DIAG, 2, QT], f16, tag="mask")
            id_sb = cp.tile([P, P], f16, tag="ident")

            nc.sync.dma_start(out=wq_sb, in_=wqT[:, :].rearrange("(c p) m -> p c m", p=P))
            nc.sync.dma_start(out=wk_sb, in_=wkT[:, :].rearrange("(c p) m -> p c m", p=P))
            nc.sync.dma_start(out=wv_sb, in_=wvT[:, :].rearrange("(c p) m -> p c m", p=P))
            nc.sync.dma_start(out=id_sb, in_=ident[:, :])

            # ---- persistent activations ----
            qT_sb = pp.tile([P, S], f16, tag="qT")
            kT_sb = pp.tile([P, S], f16, tag="kT")
            v1a = pp.tile([P, NK, 65], f16, tag="v1a")  # head 0: [v, ones]
            v1b = pp.tile([P, NK, 65], f16, tag="v1b")  # head 1
            # es double-buffered by qt parity so exp(qt) can overwrite while
            # attnV(qt-1) still reads the other buffer
            es_ev = pp.tile([P, NK - DIAG, 2, QT], f16, tag="esev")
            es_od = pp.tile([P, NK, 2, QT], f16, tag="esod")
            attnT = pp.tile([P, 2, QT], f16, tag="attnT")  # parity-buffered

            # ---- phase A: projections + RoPE + v-transposes, per 512-col chunk ----
            with (
                tc.tile_pool(name="xc", bufs=3) as xcp,
                tc.tile_pool(name="rope", bufs=2) as rp,
                tc.tile_pool(name="proj_ps", bufs=2, space="PSUM") as pps,
                tc.tile_pool(name="tp_ps", bufs=2, space="PSUM") as tpp,
            ):
                # prefetch chunk DMAs ahead of the bulky const loads
                def fetch_chunk(nt):
                    sl = slice(nt * QT, (nt + 1) * QT)
                    xc = xcp.tile([P, DC, QT], f16, tag="xc")
                    xv = xT[:, sl].rearrange("(c p) q -> p c q", p=P)
                    if nt == 0:
                        # split so the first projection matmuls start sooner
                        nc.sync.dma_start(out=xc[:, 0:2, :], in_=xv[:, 0:2, :])
                        nc.sync.dma_start(out=xc[:, 2:DC, :], in_=xv[:, 2:DC, :])
                    else:
                        nc.sync.dma_start(out=xc, in_=xv)
                    cos_c = rp.tile([P, QT], f16, tag="cosc")
                    sin_c = rp.tile([P, QT], f16, tag="sinc")
                    nc.sync.dma_start(out=cos_c, in_=cosd[:, sl])
                    nc.sync.dma_start(out=sin_c, in_=sind[:, sl])
                    return xc, cos_c, sin_c

                pref = fetch_chunk(0)
                # phase-E-only constants load behind the first x chunk
                nc.sync.dma_start(out=wo_sb, in_=woT[:, :])
                nc.sync.dma_start(out=mask_sb, in_=maskd[:, :, :, :])
                nc.sync.dma_start(out=v1a[:, :, 64:65], in_=onesv[:, :, :])
                nc.sync.dma_start(out=v1b[:, :, 64:65], in_=onesv[:, :, :])

                vt_prev = None
                for nt in range(NQ):
                    sl = slice(nt * QT, (nt + 1) * QT)
                    xc, cos_c, sin_c = pref
                    if nt + 1 < NQ:
                        pref = fetch_chunk(nt + 1)
                    psq = pps.tile([P, QT], f32, tag="psq")
                    psk = pps.tile([P, QT], f32, tag="psk")
                    psv = pps.tile([P, QT], f32, tag="psv")
                    for c in range(DC):
                        st, sp = (c == 0), (c == DC - 1)
                        nc.tensor.matmul(psq, wq_sb[:, c, :], xc[:, c, :], start=st, stop=sp)
                        nc.tensor.matmul(psk, wk_sb[:, c, :], xc[:, c, :], start=st, stop=sp)
                        nc.tensor.matmul(psv, wv_sb[:, c, :], xc[:, c, :], start=st, stop=sp)
                    vt = rp.tile([P, QT], f16, tag="vt")
                    nc.scalar.copy(qT_sb[:, sl], psq)
                    nc.scalar.copy(kT_sb[:, sl], psk)
                    nc.scalar.copy(vt, psv)
                    # RoPE on this chunk (in place); sw-mul runs on GPSIMD;
                    # q swap DMAs issue from SP, k swaps from Act (spread SEQ load)
                    for src_sb, tgname, dmaeng in (
                        (qT_sb, "swq", nc.scalar),
                        (kT_sb, "swk", nc.scalar),
                    ):
                        sw = rp.tile([P, QT], f16, tag=tgname)
                        for dst0, src0 in ((0, 32), (32, 0), (64, 96), (96, 64)):
                            dmaeng.dma_start(
                                out=sw[dst0 : dst0 + 32, :],
                                in_=src_sb[src0 : src0 + 32, sl],
                            )
                        m1 = rp.tile([P, QT], f16, tag=tgname + "m")
                        nc.vector.tensor_mul(m1, src_sb[:, sl], cos_c)
                        nc.gpsimd.tensor_mul(sw, sw, sin_c)
                        nc.vector.tensor_add(src_sb[:, sl], m1, sw)
                    # v~ transposes lag one chunk so PE never waits on the
                    # fresh vt copy
                    def vtrans(nt, vt):
                        for h, v1 in ((0, v1a), (1, v1b)):
                            hp = h * 64
                            pst = tpp.tile([P, DIAG, 64], f16, tag="pst", name="pst")
                            for j in range(DIAG):
                                nc.tensor.transpose(
                                    pst[:, j, :],
                                    vt[hp : hp + 64, j * KC : (j + 1) * KC],
                                    id_sb[hp : hp + 64, hp : hp + 64],
                                )
                            nc.vector.tensor_copy(
                                v1[:, DIAG * nt : DIAG * nt + DIAG, 0:64], pst
                            )
                    if vt_prev is not None:
                        vtrans(*vt_prev)
                    vt_prev = (nt, vt)
                if vt_prev is not None:
                    vtrans(*vt_prev)

            # ---- phase E: scores+exp, attnV, outproj, per q tile ----
            with (
                tc.tile_pool(name="sc_ps", bufs=3, space="PSUM") as scp,
                tc.tile_pool(name="att_ps", bufs=1, space="PSUM") as attp,
                tc.tile_pool(name="po_ps", bufs=1, space="PSUM") as pop,
                tc.tile_pool(name="asb", bufs=4) as asp,
                tc.tile_pool(name="yrow", bufs=1) as yrp,
            ):
                def score_unit(qt, kc):
                    q0 = qt * QT
                    es_all = es_od if qt % 2 else es_ev
                    if True:
                        j = kc - DIAG * qt  # >=0 on the diagonal
                        qoff = j * KC if j >= 0 else 0
                        ksl = slice(kc * KC, (kc + 1) * KC)
                        ps = scp.tile([P, 2, QT], f32, tag="sc")
                        nc.tensor.matmul(
                            ps[:, 0, qoff:QT],
                            kT_sb[0:64, ksl],
                            qT_sb[0:64, q0 + qoff : q0 + QT],
                            start=True, stop=True, tile_position=(0, 0),
                        )
                        nc.tensor.matmul(
                            ps[:, 1, qoff:QT],
                            kT_sb[64:128, ksl],
                            qT_sb[64:128, q0 + qoff : q0 + QT],
                            start=True, stop=True, tile_position=(64, 0),
                        )
                        if j >= 2:
                            # narrow per-head DVE trick exp (masked after)
                            for h in range(2):
                                nc.vector.tensor_scalar(
                                    out=es_all.bitcast(i16)[:, kc, h, qoff:QT],
                                    in0=ps[:, h, qoff:QT],
                                    scalar1=float(EXP_A),
                                    scalar2=float(EXP_B),
                                    op0=Alu.mult,
                                    op1=Alu.add,
                                )
                        elif j >= 0:
                            # narrow per-head exact exp
                            for h in range(2):
                                nc.scalar.activation(
                                    es_all[:, kc, h, qoff:QT],
                                    ps[:, h, qoff:QT],
                                    Act.Exp,
                                    scale=SCALE,
                                )
                        elif (qt, kc) in dve_exp:
                            nc.vector.tensor_scalar(
                                out=es_all.bitcast(i16)[:, kc, :, :].rearrange(
                                    "p h q -> p (h q)"
                                ),
                                in0=ps.rearrange("p h q -> p (h q)"),
                                scalar1=float(EXP_A),
                                scalar2=float(EXP_B),
                                op0=Alu.mult,
                                op1=Alu.add,
                            )
                        else:
                            nc.scalar.activation(
                                es_all[:, kc, :, :].rearrange("p h q -> p (h q)"),
                                ps.rearrange("p h q -> p (h q)"),
                                Act.Exp,
                                scale=SCALE,
                            )
                        if j >= 0:
                            nc.gpsimd.tensor_mul(
                                es_all[:, kc, :, qoff:QT],
                                es_all[:, kc, :, qoff:QT],
                                mask_sb[:, j, :, qoff:QT],
                            )
                def attn_units(qt):
                    """Closures for attnV matmul steps, norms, and outproj of
                    q-tile qt; interleaved between scores chunks of qt+1 so
                    the in-order PE queue always has ready work when scores
                    stall on psum banks."""
                    es_all = es_od if qt % 2 else es_ev
                    par = qt % 2
                    q0 = qt * QT
                    state = {}
                    units = []

                    def mk_av(qcl, kc, qc):
                        def f():
                            if kc == 0:
                                state[qcl] = attp.tile([P, 2, 66], f32, tag="att", name="pa")
                            pa = state[qcl]
                            st, sp = (kc == 0), (kc == qc)
                            nc.tensor.matmul(
                                pa[:, 0, 0:65],
                                es_all[:, kc, 0, qcl * KC : (qcl + 1) * KC],
                                v1a[:, kc, :],
                                start=st, stop=sp,
                            )
                            nc.tensor.matmul(
                                pa[:, 1, 0:65],
                                es_all[:, kc, 1, qcl * KC : (qcl + 1) * KC],
                                v1b[:, kc, :],
                                start=st, stop=sp,
                            )
                        return f

                    def mk_norm(qcl):
                        def f():
                            pa = state[qcl]
                            nc.vector.reciprocal(pa[:, :, 64:65], pa[:, :, 64:65])
                            asb = asp.tile([P, P], f16, tag="asb", name="asb")
                            for h in range(2):
                                nc.vector.tensor_scalar(
                                    out=asb[:, h * 64 : (h + 1) * 64],
                                    in0=pa[:, h, 0:64],
                                    scalar1=pa[:, h, 64:65],
                                    scalar2=None,
                                    op0=Alu.mult,
                                )
                            nc.sync.dma_start_transpose(
                                out=attnT[:, par, qcl * KC : (qcl + 1) * KC],
                                in_=asb,
                            )
                        return f

                    def mk_po(oc):
                        def f():
                            if oc == 0:
                                state["yrow"] = yrp.tile([P, DC, QT], f16, tag="yrow", name="yrow")
                            po = pop.tile([P, QT], f32, tag="po", name="po")
                            nc.tensor.matmul(
                                po, wo_sb[:, oc * P : (oc + 1) * P],
                                attnT[:, par, :],
                                start=True, stop=True,
                            )
                            # tail iterations have little interleave cover:
                            # alternate the psum drain across Act/DVE there
                            if qt >= NQ - 2 and oc % 2:
                                nc.scalar.copy(state["yrow"][:, oc, :], po)
                            else:
                                nc.vector.tensor_copy(state["yrow"][:, oc, :], po)
                            if oc == DC - 1:
                                qsl = slice(q0, q0 + QT)
                                nc.scalar.dma_start(
                                    out=yT[:, qsl].rearrange("(c p) q -> p c q", p=P),
                                    in_=state["yrow"],
                                )
                        return f

                    av_units = []
                    po_units = []
                    for qcl in range(DIAG):
                        qc = DIAG * qt + qcl
                        for kc in range(qc + 1):
                            av_units.append(mk_av(qcl, kc, qc))
                        av_units.append(mk_norm(qcl))
                    for oc in range(DC):
                        po_units.append(mk_po(oc))
                    return av_units, po_units

                # software pipeline: attnV/norm of qt-1 and outproj of qt-2
                # interleave between the scores chunks of qt (proportional
                # merge) so the in-order PE queue always has ready work
                avpo = {}
                for it in range(NQ + 2):
                    units = []
                    if 1 <= it <= NQ:
                        avpo[it - 1] = attn_units(it - 1)
                        units += avpo[it - 1][0]
                    if it >= 2:
                        units += avpo.pop(it - 2)[1]
                    if it < NQ:
                        nkc = DIAG * it + DIAG
                        done = 0
                        for kc in range(nkc):
                            score_unit(it, kc)
                            want = (kc + 1) * len(units) // nkc
                            while done < want:
                                units[done]()
                                done += 1
                    else:
                        # tail: interleave attnV and outproj streams
                        a, b = (avpo[it - 1][0], units[len(avpo[it - 1][0]):]) if (
                            it - 1
                        ) in avpo else (units, [])
                        ia = ib = 0
                        n = max(len(a), 1)
                        for i_ in range(len(a)):
                            a[i_]()
                            want = (i_ + 1) * len(b) // n
                            while ib < want:
                                b[ib]()
                                ib += 1
                        while ib < len(b):
                            b[ib]()
                            ib += 1

    nc.compile()
    _BUILD_CACHE[key] = nc
    return nc


def host_prep(x, Wq, Wk, Wv, Wo, S=S_FULL):
    """Build per-core input maps (numpy, fp16)."""
    x = np.asarray(x, np.float32).reshape(S, D)
    xT = np.ascontiguousarray(x.T, dtype=np.float16)

    perm64 = np.concatenate([np.arange(0, 64, 2), np.arange(1, 64, 2)])
    j32 = np.arange(32, dtype=np.float64)
    rates = THETA ** (-2.0 * j32 / DK)
    pos = np.arange(S, dtype=np.float64)
    ang = rates[:, None] * pos[None, :]  # (32, S)
    cos32 = np.cos(ang)
    sin32 = np.sin(ang)
    cosd = np.tile(cos32, (4, 1)).astype(np.float16)  # (128, S)
    signs = np.repeat([-1.0, 1.0, -1.0, 1.0], 32)[:, None]
    sind = (np.tile(sin32, (4, 1)) * signs).astype(np.float16)

    DIAG = QT // KC
    r = np.arange(P)[:, None, None, None]
    jj = np.arange(DIAG)[None, :, None, None]
    q_local = np.arange(QT)[None, None, None, :]
    maskd = np.broadcast_to(
        (q_local >= jj * KC + r), (P, DIAG, 2, QT)
    ).astype(np.float16)

    ident = np.eye(P, dtype=np.float16)

    in_maps = []
    for g in range(N_CORES):
        h0, h1 = 2 * g, 2 * g + 1
        idx_qk = np.concatenate([h0 * DK + perm64, h1 * DK + perm64])
        idx_v = np.arange(h0 * DK, h0 * DK + 2 * DK)
        in_maps.append(
            {
                "xT": xT,
                "wqT": np.ascontiguousarray(np.asarray(Wq)[idx_qk, :].T, dtype=np.float16),
                "wkT": np.ascontiguousarray(np.asarray(Wk)[idx_qk, :].T, dtype=np.float16),
                "wvT": np.ascontiguousarray(np.asarray(Wv)[idx_v, :].T, dtype=np.float16),
                "woT": np.ascontiguousarray(np.asarray(Wo)[:, idx_v].T, dtype=np.float16),
                "cosd": cosd,
                "sind": sind,
                "maskd": maskd,
                "ident": ident,
                "onesv": np.ones((P, S // KC, 1), np.float16),
            }
        )
    return in_maps


def run_cores(x, Wq, Wk, Wv, Wo, S=S_FULL, core_ids=None, trace=False):
    from concourse.bass_utils import run_bass_kernel_spmd

    nc = build(S)
    in_maps = host_prep(x, Wq, Wk, Wv, Wo, S=S)
    if core_ids is None:
        core_ids = list(range(N_CORES))
    in_maps = in_maps[: len(core_ids)]
    res = run_bass_kernel_spmd(nc, in_maps, core_ids, trace=trace)
    return res


def kernel(x, Wq, Wk, Wv, Wo):
    x = np.asarray(x, np.float32)
    res = run_cores(x, np.asarray(Wq), np.asarray(Wk), np.asarray(Wv), np.asarray(Wo))
    y = np.zeros((D, S_FULL), np.float64)
    for r in res.results:
        y += r["yT"].astype(np.float64)
    return np.ascontiguousarray(y.T, dtype=np.float32).reshape(1, S_FULL, D)


# revision 35
# speedup vs baseline: 1.0545x; 1.0157x over previous
"""Multi-head causal self-attention with RoPE on 8 Trainium2 NeuronCores.

Sharding: 16 heads -> 8 cores (2 heads/core, head/tensor parallel).
Wq/Wk/Wv column-sharded (per-head-group rows of W), Wo row-sharded.
Each core computes a full (S, D) partial of the output projection in fp16;
the host sums the 8 partials (the row-parallel reduce).

v2 layout notes (cost-model driven):
 - everything fp16 on SBUF; psum fp32; moving matmul operands fp16 so every
   matmul runs at 1 cycle/column regardless of width.
 - attnV computed in [q_part, dk_free] layout: per 128-query subchunk an
   accumulating psum [128, 2*66] ([v|Z] per head), with es chunks as the
   stationary operand -> 65-column matmuls instead of 512-column ones.
 - softmax normalization: per-partition reciprocal of the Z column + one
   tensor_scalar multiply per head -> fp16 attn tile, then a DMA-transpose
   (14ns/tile) back to [dk, q] for the output projection.
 - exp work split: Activation engine does most chunks (exact exp); ~1/3 of
   the off-diagonal chunks run on DVE via a Schraudolph int16 bit-trick
   (out_i16 = round(s*A + B) bitcast fp16 ~= exp(s/8), rel err ~2%).
 - causal diagonal chunks compute only the valid score columns (narrow
   matmuls + narrow per-head exp) and are masked on GPSIMD.
 - output projection psums drain via DVE/Act copies into a per-qtile fp16
   row buffer, one DMA per qtile to DRAM.
"""

import sys

for _p in ("/opt/trn_rl_repo", "/root/.axon_site/_ro/trn_rl_repo"):
    if _p not in sys.path:
        sys.path.insert(0, _p)

import numpy as np

S_FULL = 4096
D = 1024
NH = 16
DK = 64
P = 128
QT = 512  # q tile (free dim of score tiles)
KC = 128  # k chunk (partition dim of score tiles)
DC = D // P  # 8 contraction chunks for the projections
THETA = 10000.0
N_CORES = 8

# Schraudolph exp constants for fp16 bit pattern: round(s*A + B) ~ fp16(exp(s/8))
LOG2E = 1.4426950408889634
EXP_A = (1 << 10) * LOG2E * 0.125
EXP_B = 15.0 * (1 << 10) - 58.9

_BUILD_CACHE: dict = {}


def _dve_exp_set(S: int) -> set:
    """(qt, kc) pairs whose exp runs on DVE via the bit trick (off-diagonal
    chunks only; roughly 1/4 of them, interleaved)."""
    NQ = S // QT
    DIAG = QT // KC
    out = set()
    idx = 0
    for qt in range(NQ):
        for kc in range(DIAG * qt):  # off-diagonal only
            if idx % 16 < 4:
                out.add((qt, kc))
            idx += 1
    return out


def build(S: int = S_FULL, reps: int = 1, debug_taps: bool = False):
    """Build the per-core Bass program (same program for all cores)."""
    key = (S, reps, debug_taps)
    if key in _BUILD_CACHE:
        return _BUILD_CACHE[key]

    import concourse.bacc as bacc
    import concourse.tile as tile
    from concourse import mybir

    f32 = mybir.dt.float32
    f16 = mybir.dt.float16
    i16 = mybir.dt.int16
    Alu = mybir.AluOpType
    Act = mybir.ActivationFunctionType

    NQ = S // QT
    NK = S // KC
    DIAG = QT // KC  # k-chunks per q-tile on the diagonal (4)
    dve_exp = _dve_exp_set(S)
    SCALE = float(DK) ** -0.5

    nc = bacc.Bacc(
        "TRN2", target_bir_lowering=False, debug=False, num_devices=N_CORES
    )
    xT = nc.dram_tensor("xT", [D, S], f16, kind="ExternalInput")
    wqT = nc.dram_tensor("wqT", [D, P], f16, kind="ExternalInput")
    wkT = nc.dram_tensor("wkT", [D, P], f16, kind="ExternalInput")
    wvT = nc.dram_tensor("wvT", [D, P], f16, kind="ExternalInput")
    woT = nc.dram_tensor("woT", [P, D], f16, kind="ExternalInput")
    cosd = nc.dram_tensor("cosd", [P, S], f16, kind="ExternalInput")
    sind = nc.dram_tensor("sind", [P, S], f16, kind="ExternalInput")
    maskd = nc.dram_tensor("maskd", [P, DIAG, 2, QT], f16, kind="ExternalInput")
    ident = nc.dram_tensor("ident", [P, P], f16, kind="ExternalInput")
    onesv = nc.dram_tensor("onesv", [P, NK, 1], f16, kind="ExternalInput")
    yT = nc.dram_tensor("yT", [D, S], f16, kind="ExternalOutput")
    if debug_taps:
        NKt = S // KC
        qT_out = nc.dram_tensor("qT_out", [P, S], f16, kind="ExternalOutput")
        kT_out = nc.dram_tensor("kT_out", [P, S], f16, kind="ExternalOutput")
        v1a_out = nc.dram_tensor("v1a_out", [P, NKt, 65], f16, kind="ExternalOutput")
        es_out = nc.dram_tensor("es_out", [P, NKt, 2, QT], f16, kind="ExternalOutput")
        at_out = nc.dram_tensor("at_out", [P, 2, QT], f16, kind="ExternalOutput")
        pa_out = nc.dram_tensor("pa_out", [P, 2, 66], f32, kind="ExternalOutput")
        asb_out = nc.dram_tensor("asb_out", [P, P], f16, kind="ExternalOutput")

    with tile.TileContext(nc) as tc:
        with (
            tc.tile_pool(name="const", bufs=1) as cp,
            tc.tile_pool(name="persist", bufs=1) as pp,
        ):
            # ---- constants ----
            wq_sb = cp.tile([P, DC, P], f16, tag="wq")
            wk_sb = cp.tile([P, DC, P], f16, tag="wk")
            wv_sb = cp.tile([P, DC, P], f16, tag="wv")
            wo_sb = cp.tile([P, D], f16, tag="wo")
            mask_sb = cp.tile([P, DIAG, 2, QT], f16, tag="mask")
            id_sb = cp.tile([P, P], f16, tag="ident")

            nc.sync.dma_start(out=wq_sb, in_=wqT[:, :].rearrange("(c p) m -> p c m", p=P))
            nc.sync.dma_start(out=wk_sb, in_=wkT[:, :].rearrange("(c p) m -> p c m", p=P))
            nc.sync.dma_start(out=wv_sb, in_=wvT[:, :].rearrange("(c p) m -> p c m", p=P))
            nc.sync.dma_start(out=id_sb, in_=ident[:, :])

            # ---- persistent activations ----
            qT_sb = pp.tile([P, S], f16, tag="qT")
            kT_sb = pp.tile([P, S], f16, tag="kT")
            v1a = pp.tile([P, NK, 65], f16, tag="v1a")  # head 0: [v, ones]
            v1b = pp.tile([P, NK, 65], f16, tag="v1b")  # head 1
            # es double-buffered by qt parity so exp(qt) can overwrite while
            # attnV(qt-1) still reads the other buffer
            es_ev = pp.tile([P, NK - DIAG, 2, QT], f16, tag="esev")
            es_od = pp.tile([P, NK, 2, QT], f16, tag="esod")
            attnT = pp.tile([P, 2, QT], f16, tag="attnT")  # parity-buffered

            # ---- phase A: projections + RoPE + v-transposes, per 512-col chunk ----
            with (
                tc.tile_pool(name="xc", bufs=3) as xcp,
                tc.tile_pool(name="rope", bufs=2) as rp,
                tc.tile_pool(name="proj_ps", bufs=2, space="PSUM") as pps,
                tc.tile_pool(name="tp_ps", bufs=2, space="PSUM") as tpp,
            ):
                # prefetch chunk DMAs ahead of the bulky const loads
                def fetch_chunk(nt):
                    sl = slice(nt * QT, (nt + 1) * QT)
                    xc = xcp.tile([P, DC, QT], f16, tag="xc")
                    xv = xT[:, sl].rearrange("(c p) q -> p c q", p=P)
                    if nt == 0:
                        # split so the first projection matmuls start sooner
                        nc.sync.dma_start(out=xc[:, 0:2, :], in_=xv[:, 0:2, :])
                        nc.sync.dma_start(out=xc[:, 2:DC, :], in_=xv[:, 2:DC, :])
                    else:
                        nc.sync.dma_start(out=xc, in_=xv)
                    cos_c = rp.tile([P, QT], f16, tag="cosc")
                    sin_c = rp.tile([P, QT], f16, tag="sinc")
                    nc.sync.dma_start(out=cos_c, in_=cosd[:, sl])
                    nc.sync.dma_start(out=sin_c, in_=sind[:, sl])
                    return xc, cos_c, sin_c

                pref = fetch_chunk(0)
                # phase-E-only constants load behind the first x chunk
                nc.sync.dma_start(out=wo_sb, in_=woT[:, :])
                nc.sync.dma_start(out=mask_sb, in_=maskd[:, :, :, :])
                nc.sync.dma_start(out=v1a[:, :, 64:65], in_=onesv[:, :, :])
                nc.sync.dma_start(out=v1b[:, :, 64:65], in_=onesv[:, :, :])

                vt_prev = None
                for nt in range(NQ):
                    sl = slice(nt * QT, (nt + 1) * QT)
                    xc, cos_c, sin_c = pref
                    if nt + 1 < NQ:
                        pref = fetch_chunk(nt + 1)
                    psq = pps.tile([P, QT], f32, tag="psq")
                    psk = pps.tile([P, QT], f32, tag="psk")
                    psv = pps.tile([P, QT], f32, tag="psv")
                    for c in range(DC):
                        st, sp = (c == 0), (c == DC - 1)
                        nc.tensor.matmul(psq, wq_sb[:, c, :], xc[:, c, :], start=st, stop=sp)
                        nc.tensor.matmul(psk, wk_sb[:, c, :], xc[:, c, :], start=st, stop=sp)
                        nc.tensor.matmul(psv, wv_sb[:, c, :], xc[:, c, :], start=st, stop=sp)
                    vt = rp.tile([P, QT], f16, tag="vt")
                    nc.scalar.copy(qT_sb[:, sl], psq)
                    nc.scalar.copy(kT_sb[:, sl], psk)
                    nc.scalar.copy(vt, psv)
                    # RoPE on this chunk (in place); sw-mul runs on GPSIMD;
                    # q swap DMAs issue from SP, k swaps from Act (spread SEQ load)
                    for src_sb, tgname, dmaeng in (
                        (qT_sb, "swq", nc.scalar),
                        (kT_sb, "swk", nc.scalar),
                    ):
                        sw = rp.tile([P, QT], f16, tag=tgname)
                        for dst0, src0 in ((0, 32), (32, 0), (64, 96), (96, 64)):
                            dmaeng.dma_start(
                                out=sw[dst0 : dst0 + 32, :],
                                in_=src_sb[src0 : src0 + 32, sl],
                            )
                        m1 = rp.tile([P, QT], f16, tag=tgname + "m")
                        nc.vector.tensor_mul(m1, src_sb[:, sl], cos_c)
                        nc.gpsimd.tensor_mul(sw, sw, sin_c)
                        nc.gpsimd.tensor_add(src_sb[:, sl], m1, sw)
                    # v~ transposes lag one chunk so PE never waits on the
                    # fresh vt copy
                    def vtrans(nt, vt):
                        for h, v1 in ((0, v1a), (1, v1b)):
                            hp = h * 64
                            pst = tpp.tile([P, DIAG, 64], f16, tag="pst", name="pst")
                            for j in range(DIAG):
                                nc.tensor.transpose(
                                    pst[:, j, :],
                                    vt[hp : hp + 64, j * KC : (j + 1) * KC],
                                    id_sb[hp : hp + 64, hp : hp + 64],
                                )
                            nc.vector.tensor_copy(
                                v1[:, DIAG * nt : DIAG * nt + DIAG, 0:64], pst
                            )
                    if vt_prev is not None:
                        vtrans(*vt_prev)
                    vt_prev = (nt, vt)
                if vt_prev is not None:
                    vtrans(*vt_prev)

            # ---- phase E: scores+exp, attnV, outproj, per q tile ----
            with (
                tc.tile_pool(name="sc_ps", bufs=3, space="PSUM") as scp,
                tc.tile_pool(name="att_ps", bufs=1, space="PSUM") as attp,
                tc.tile_pool(name="po_ps", bufs=1, space="PSUM") as pop,
                tc.tile_pool(name="asb", bufs=4) as asp,
                tc.tile_pool(name="yrow", bufs=1) as yrp,
            ):
                def score_unit(qt, kc):
                    q0 = qt * QT
                    es_all = es_od if qt % 2 else es_ev
                    if True:
                        j = kc - DIAG * qt  # >=0 on the diagonal
                        qoff = j * KC if j >= 0 else 0
                        ksl = slice(kc * KC, (kc + 1) * KC)
                        ps = scp.tile([P, 2, QT], f32, tag="sc")
                        diag = j >= 0
                        if diag:
                            # causal mask folded in as a -1e4 bias via an
                            # identity matmul into the psum (per head)
                            for h in range(2):
                                nc.tensor.matmul(
                                    ps[:, h, qoff:QT],
                                    id_sb,
                                    mask_sb[:, j, h, qoff:QT],
                                    start=True, stop=False,
                                )
                        nc.tensor.matmul(
                            ps[:, 0, qoff:QT],
                            kT_sb[0:64, ksl],
                            qT_sb[0:64, q0 + qoff : q0 + QT],
                            start=not diag, stop=True, tile_position=(0, 0),
                        )
                        nc.tensor.matmul(
                            ps[:, 1, qoff:QT],
                            kT_sb[64:128, ksl],
                            qT_sb[64:128, q0 + qoff : q0 + QT],
                            start=not diag, stop=True, tile_position=(64, 0),
                        )
                        if j >= 2:
                            # narrow per-head DVE trick exp (masked after)
                            for h in range(2):
                                nc.vector.tensor_scalar(
                                    out=es_all.bitcast(i16)[:, kc, h, qoff:QT],
                                    in0=ps[:, h, qoff:QT],
                                    scalar1=float(EXP_A),
                                    scalar2=float(EXP_B),
                                    op0=Alu.mult,
                                    op1=Alu.add,
                                )
                        elif j >= 0:
                            # narrow per-head exact exp
                            for h in range(2):
                                nc.scalar.activation(
                                    es_all[:, kc, h, qoff:QT],
                                    ps[:, h, qoff:QT],
                                    Act.Exp,
                                    scale=SCALE,
                                )
                        elif (qt, kc) in dve_exp:
                            nc.vector.tensor_scalar(
                                out=es_all.bitcast(i16)[:, kc, :, :].rearrange(
                                    "p h q -> p (h q)"
                                ),
                                in0=ps.rearrange("p h q -> p (h q)"),
                                scalar1=float(EXP_A),
                                scalar2=float(EXP_B),
                                op0=Alu.mult,
                                op1=Alu.add,
                            )
                        else:
                            nc.scalar.activation(
                                es_all[:, kc, :, :].rearrange("p h q -> p (h q)"),
                                ps.rearrange("p h q -> p (h q)"),
                                Act.Exp,
                                scale=SCALE,
                            )
                def attn_units(qt):
                    """Closures for attnV matmul steps, norms, and outproj of
                    q-tile qt; interleaved between scores chunks of qt+1 so
                    the in-order PE queue always has ready work when scores
                    stall on psum banks."""
                    es_all = es_od if qt % 2 else es_ev
                    par = qt % 2
                    q0 = qt * QT
                    state = {}
                    units = []

                    def mk_av(qcl, kc, qc):
                        def f():
                            if kc == 0:
                                state[qcl] = attp.tile([P, 2, 66], f32, tag="att", name="pa")
                            pa = state[qcl]
                            # one psum group across both heads: start=True
                            # clears has_written for the whole 2KB zero
                            # region, so only the very first matmul may start
                            nc.tensor.matmul(
                                pa[:, 0, 0:65],
                                es_all[:, kc, 0, qcl * KC : (qcl + 1) * KC],
                                v1a[:, kc, :],
                                start=(kc == 0), stop=False,
                            )
                            nc.tensor.matmul(
                                pa[:, 1, 0:65],
                                es_all[:, kc, 1, qcl * KC : (qcl + 1) * KC],
                                v1b[:, kc, :],
                                start=False, stop=(kc == qc),
                            )
                        return f

                    def mk_norm(qcl):
                        def f():
                            pa = state[qcl]
                            if debug_taps and qt == NQ - 1 and qcl == DIAG - 1:
                                pacp = asp.tile([P, 2, 66], f32, tag="pacp", name="pacp")
                                nc.vector.tensor_copy(pacp, pa)
                                nc.sync.dma_start(out=pa_out[:, :, :], in_=pacp)
                            nc.vector.reciprocal(pa[:, :, 64:65], pa[:, :, 64:65])
                            asb = asp.tile([P, P], f16, tag="asb", name="asb")
                            for h in range(2):
                                nc.vector.tensor_scalar(
                                    out=asb[:, h * 64 : (h + 1) * 64],
                                    in0=pa[:, h, 0:64],
                                    scalar1=pa[:, h, 64:65],
                                    scalar2=None,
                                    op0=Alu.mult,
                                )
                            nc.sync.dma_start_transpose(
                                out=attnT[:, par, qcl * KC : (qcl + 1) * KC],
                                in_=asb,
                            )
                            if debug_taps and qt == NQ - 1 and qcl == DIAG - 1:
                                nc.sync.dma_start(out=asb_out[:, :], in_=asb)
                        return f

                    def mk_po(oc):
                        def f():
                            if oc == 0:
                                state["yrow"] = yrp.tile([P, DC, QT], f16, tag="yrow", name="yrow")
                            po = pop.tile([P, QT], f32, tag="po", name="po")
                            nc.tensor.matmul(
                                po, wo_sb[:, oc * P : (oc + 1) * P],
                                attnT[:, par, :],
                                start=True, stop=True,
                            )
                            # tail iterations have little interleave cover:
                            # alternate the psum drain across Act/DVE there
                            if qt >= NQ - 2 and oc % 2:
                                nc.scalar.copy(state["yrow"][:, oc, :], po)
                            else:
                                nc.vector.tensor_copy(state["yrow"][:, oc, :], po)
                            if oc == DC - 1:
                                qsl = slice(q0, q0 + QT)
                                nc.scalar.dma_start(
                                    out=yT[:, qsl].rearrange("(c p) q -> p c q", p=P),
                                    in_=state["yrow"],
                                )
                        return f

                    av_units = []
                    po_units = []
                    for qcl in range(DIAG):
                        qc = DIAG * qt + qcl
                        for kc in range(qc + 1):
                            av_units.append(mk_av(qcl, kc, qc))
                        av_units.append(mk_norm(qcl))
                    for oc in range(DC):
                        po_units.append(mk_po(oc))
                    return av_units, po_units

                # software pipeline: attnV/norm of qt-1 and outproj of qt-2
                # interleave between the scores chunks of qt (proportional
                # merge) so the in-order PE queue always has ready work
                avpo = {}
                for it in range(NQ + 2):
                    units = []
                    if 1 <= it <= NQ:
                        avpo[it - 1] = attn_units(it - 1)
                        units += avpo[it - 1][0]
                    if it >= 2:
                        units += avpo.pop(it - 2)[1]
                    if it < NQ:
                        nkc = DIAG * it + DIAG
                        done = 0
                        for kc in range(nkc):
                            score_unit(it, kc)
                            want = (kc + 1) * len(units) // nkc
                            while done < want:
                                units[done]()
                                done += 1
                    else:
                        # tail: interleave attnV and outproj streams
                        a, b = (avpo[it - 1][0], units[len(avpo[it - 1][0]):]) if (
                            it - 1
                        ) in avpo else (units, [])
                        ia = ib = 0
                        n = max(len(a), 1)
                        for i_ in range(len(a)):
                            a[i_]()
                            want = (i_ + 1) * len(b) // n
                            while ib < want:
                                b[ib]()
                                ib += 1
                        while ib < len(b):
                            b[ib]()
                            ib += 1
                if debug_taps:
                    nc.sync.dma_start(out=qT_out[:, :], in_=qT_sb)
                    nc.sync.dma_start(out=kT_out[:, :], in_=kT_sb)
                    nc.sync.dma_start(out=v1a_out[:, :, :], in_=v1a)
                    nc.sync.dma_start(out=es_out[:, :, :, :], in_=es_od)
                    nc.sync.dma_start(out=at_out[:, :, :], in_=attnT)

    nc.compile()
    _BUILD_CACHE[key] = nc
    return nc


def host_prep(x, Wq, Wk, Wv, Wo, S=S_FULL):
    """Build per-core input maps (numpy, fp16)."""
    x = np.asarray(x, np.float32).reshape(S, D)
    xT = np.ascontiguousarray(x.T, dtype=np.float16)

    perm64 = np.concatenate([np.arange(0, 64, 2), np.arange(1, 64, 2)])
    j32 = np.arange(32, dtype=np.float64)
    rates = THETA ** (-2.0 * j32 / DK)
    pos = np.arange(S, dtype=np.float64)
    ang = rates[:, None] * pos[None, :]  # (32, S)
    cos32 = np.cos(ang)
    sin32 = np.sin(ang)
    cosd = np.tile(cos32, (4, 1)).astype(np.float16)  # (128, S)
    signs = np.repeat([-1.0, 1.0, -1.0, 1.0], 32)[:, None]
    sind = (np.tile(sin32, (4, 1)) * signs).astype(np.float16)

    DIAG = QT // KC
    r = np.arange(P)[:, None, None, None]
    jj = np.arange(DIAG)[None, :, None, None]
    q_local = np.arange(QT)[None, None, None, :]
    maskd = np.broadcast_to(
        np.where(q_local >= jj * KC + r, 0.0, -10000.0), (P, DIAG, 2, QT)
    ).astype(np.float16)

    ident = np.eye(P, dtype=np.float16)

    in_maps = []
    for g in range(N_CORES):
        h0, h1 = 2 * g, 2 * g + 1
        idx_qk = np.concatenate([h0 * DK + perm64, h1 * DK + perm64])
        idx_v = np.arange(h0 * DK, h0 * DK + 2 * DK)
        in_maps.append(
            {
                "xT": xT,
                "wqT": np.ascontiguousarray(np.asarray(Wq)[idx_qk, :].T, dtype=np.float16),
                "wkT": np.ascontiguousarray(np.asarray(Wk)[idx_qk, :].T, dtype=np.float16),
                "wvT": np.ascontiguousarray(np.asarray(Wv)[idx_v, :].T, dtype=np.float16),
                "woT": np.ascontiguousarray(np.asarray(Wo)[:, idx_v].T, dtype=np.float16),
                "cosd": cosd,
                "sind": sind,
                "maskd": maskd,
                "ident": ident,
                "onesv": np.ones((P, S // KC, 1), np.float16),
            }
        )
    return in_maps


def run_cores(x, Wq, Wk, Wv, Wo, S=S_FULL, core_ids=None, trace=False):
    from concourse.bass_utils import run_bass_kernel_spmd

    nc = build(S)
    in_maps = host_prep(x, Wq, Wk, Wv, Wo, S=S)
    if core_ids is None:
        core_ids = list(range(N_CORES))
    in_maps = in_maps[: len(core_ids)]
    res = run_bass_kernel_spmd(nc, in_maps, core_ids, trace=trace)
    return res


def kernel(x, Wq, Wk, Wv, Wo):
    x = np.asarray(x, np.float32)
    res = run_cores(x, np.asarray(Wq), np.asarray(Wk), np.asarray(Wv), np.asarray(Wo))
    y = np.zeros((D, S_FULL), np.float64)
    for r in res.results:
        y += r["yT"].astype(np.float64)
    return np.ascontiguousarray(y.T, dtype=np.float32).reshape(1, S_FULL, D)


# revision 36
# speedup vs baseline: 1.0618x; 1.0070x over previous
"""Multi-head causal self-attention with RoPE on 8 Trainium2 NeuronCores.

Sharding: 16 heads -> 8 cores (2 heads/core, head/tensor parallel).
Wq/Wk/Wv column-sharded (per-head-group rows of W), Wo row-sharded.
Each core computes a full (S, D) partial of the output projection in fp16;
the host sums the 8 partials (the row-parallel reduce).

v2 layout notes (cost-model driven):
 - everything fp16 on SBUF; psum fp32; moving matmul operands fp16 so every
   matmul runs at 1 cycle/column regardless of width.
 - attnV computed in [q_part, dk_free] layout: per 128-query subchunk an
   accumulating psum [128, 2*66] ([v|Z] per head), with es chunks as the
   stationary operand -> 65-column matmuls instead of 512-column ones.
 - softmax normalization: per-partition reciprocal of the Z column + one
   tensor_scalar multiply per head -> fp16 attn tile, then a DMA-transpose
   (14ns/tile) back to [dk, q] for the output projection.
 - exp work split: Activation engine does most chunks (exact exp); ~1/3 of
   the off-diagonal chunks run on DVE via a Schraudolph int16 bit-trick
   (out_i16 = round(s*A + B) bitcast fp16 ~= exp(s/8), rel err ~2%).
 - causal diagonal chunks compute only the valid score columns (narrow
   matmuls + narrow per-head exp) and are masked on GPSIMD.
 - output projection psums drain via DVE/Act copies into a per-qtile fp16
   row buffer, one DMA per qtile to DRAM.
"""

import sys

for _p in ("/opt/trn_rl_repo", "/root/.axon_site/_ro/trn_rl_repo"):
    if _p not in sys.path:
        sys.path.insert(0, _p)

import numpy as np

S_FULL = 4096
D = 1024
NH = 16
DK = 64
P = 128
QT = 512  # q tile (free dim of score tiles)
KC = 128  # k chunk (partition dim of score tiles)
DC = D // P  # 8 contraction chunks for the projections
THETA = 10000.0
N_CORES = 8

# Schraudolph exp constants for fp16 bit pattern: round(s*A + B) ~ fp16(exp(s/8))
LOG2E = 1.4426950408889634
EXP_A = (1 << 10) * LOG2E * 0.125
EXP_B = 15.0 * (1 << 10) - 58.9

_BUILD_CACHE: dict = {}


def _dve_exp_set(S: int) -> set:
    """(qt, kc) pairs whose exp runs on DVE via the bit trick (off-diagonal
    chunks only; roughly 1/4 of them, interleaved)."""
    NQ = S // QT
    DIAG = QT // KC
    out = set()
    idx = 0
    for qt in range(NQ):
        for kc in range(DIAG * qt):  # off-diagonal only
            if idx % 16 < 4:
                out.add((qt, kc))
            idx += 1
    return out


def build(S: int = S_FULL, reps: int = 1, debug_taps: bool = False):
    """Build the per-core Bass program (same program for all cores)."""
    key = (S, reps, debug_taps)
    if key in _BUILD_CACHE:
        return _BUILD_CACHE[key]

    import concourse.bacc as bacc
    import concourse.tile as tile
    from concourse import mybir

    f32 = mybir.dt.float32
    f16 = mybir.dt.float16
    i16 = mybir.dt.int16
    Alu = mybir.AluOpType
    Act = mybir.ActivationFunctionType

    NQ = S // QT
    NK = S // KC
    DIAG = QT // KC  # k-chunks per q-tile on the diagonal (4)
    dve_exp = _dve_exp_set(S)
    SCALE = float(DK) ** -0.5

    nc = bacc.Bacc(
        "TRN2", target_bir_lowering=False, debug=False, num_devices=N_CORES
    )
    xT = nc.dram_tensor("xT", [D, S], f16, kind="ExternalInput")
    wqT = nc.dram_tensor("wqT", [D, P], f16, kind="ExternalInput")
    wkT = nc.dram_tensor("wkT", [D, P], f16, kind="ExternalInput")
    wvT = nc.dram_tensor("wvT", [D, P], f16, kind="ExternalInput")
    woT = nc.dram_tensor("woT", [P, D], f16, kind="ExternalInput")
    cosd = nc.dram_tensor("cosd", [P, S], f16, kind="ExternalInput")
    sind = nc.dram_tensor("sind", [P, S], f16, kind="ExternalInput")
    maskd = nc.dram_tensor("maskd", [P, DIAG, 2, QT], f16, kind="ExternalInput")
    ident = nc.dram_tensor("ident", [P, P], f16, kind="ExternalInput")
    onesv = nc.dram_tensor("onesv", [P, NK, 1], f16, kind="ExternalInput")
    yT = nc.dram_tensor("yT", [D, S], f16, kind="ExternalOutput")
    if debug_taps:
        NKt = S // KC
        qT_out = nc.dram_tensor("qT_out", [P, S], f16, kind="ExternalOutput")
        kT_out = nc.dram_tensor("kT_out", [P, S], f16, kind="ExternalOutput")
        v1a_out = nc.dram_tensor("v1a_out", [P, NKt, 65], f16, kind="ExternalOutput")
        es_out = nc.dram_tensor("es_out", [P, NKt, 2, QT], f16, kind="ExternalOutput")
        at_out = nc.dram_tensor("at_out", [P, 2, QT], f16, kind="ExternalOutput")
        pa_out = nc.dram_tensor("pa_out", [P, 2, 66], f32, kind="ExternalOutput")
        asb_out = nc.dram_tensor("asb_out", [P, P], f16, kind="ExternalOutput")

    with tile.TileContext(nc) as tc:
        with (
            tc.tile_pool(name="const", bufs=1) as cp,
            tc.tile_pool(name="persist", bufs=1) as pp,
        ):
            # ---- constants ----
            wq_sb = cp.tile([P, DC, P], f16, tag="wq")
            wk_sb = cp.tile([P, DC, P], f16, tag="wk")
            wv_sb = cp.tile([P, DC, P], f16, tag="wv")
            wo_sb = cp.tile([P, D], f16, tag="wo")
            mask_sb = cp.tile([P, DIAG, 2, QT], f16, tag="mask")
            id_sb = cp.tile([P, P], f16, tag="ident")

            nc.sync.dma_start(out=wq_sb, in_=wqT[:, :].rearrange("(c p) m -> p c m", p=P))
            nc.sync.dma_start(out=wk_sb, in_=wkT[:, :].rearrange("(c p) m -> p c m", p=P))
            nc.sync.dma_start(out=wv_sb, in_=wvT[:, :].rearrange("(c p) m -> p c m", p=P))
            nc.sync.dma_start(out=id_sb, in_=ident[:, :])

            # ---- persistent activations ----
            qT_sb = pp.tile([P, S], f16, tag="qT")
            kT_sb = pp.tile([P, S], f16, tag="kT")
            v1a = pp.tile([P, NK, 65], f16, tag="v1a")  # head 0: [v, ones]
            v1b = pp.tile([P, NK, 65], f16, tag="v1b")  # head 1
            # es double-buffered by qt parity so exp(qt) can overwrite while
            # attnV(qt-1) still reads the other buffer
            es_ev = pp.tile([P, NK - DIAG, 2, QT], f16, tag="esev")
            es_od = pp.tile([P, NK, 2, QT], f16, tag="esod")
            attnT = pp.tile([P, 2, QT], f16, tag="attnT")  # parity-buffered

            # ---- phase A: projections + RoPE + v-transposes, per 512-col chunk ----
            with (
                tc.tile_pool(name="xc", bufs=3) as xcp,
                tc.tile_pool(name="rope", bufs=2) as rp,
                tc.tile_pool(name="proj_ps", bufs=2, space="PSUM") as pps,
                tc.tile_pool(name="tp_ps", bufs=2, space="PSUM") as tpp,
            ):
                # prefetch chunk DMAs ahead of the bulky const loads
                def fetch_chunk(nt):
                    sl = slice(nt * QT, (nt + 1) * QT)
                    xc = xcp.tile([P, DC, QT], f16, tag="xc")
                    xv = xT[:, sl].rearrange("(c p) q -> p c q", p=P)
                    if nt == 0:
                        # split so the first projection matmuls start sooner
                        nc.sync.dma_start(out=xc[:, 0:2, :], in_=xv[:, 0:2, :])
                        nc.sync.dma_start(out=xc[:, 2:DC, :], in_=xv[:, 2:DC, :])
                    else:
                        nc.sync.dma_start(out=xc, in_=xv)
                    cos_c = rp.tile([P, QT], f16, tag="cosc")
                    sin_c = rp.tile([P, QT], f16, tag="sinc")
                    nc.sync.dma_start(out=cos_c, in_=cosd[:, sl])
                    nc.sync.dma_start(out=sin_c, in_=sind[:, sl])
                    return xc, cos_c, sin_c

                pref = fetch_chunk(0)
                # phase-E-only constants load behind the first x chunk
                nc.sync.dma_start(out=wo_sb, in_=woT[:, :])
                nc.sync.dma_start(out=mask_sb, in_=maskd[:, :, :, :])
                nc.sync.dma_start(out=v1a[:, :, 64:65], in_=onesv[:, :, :])
                nc.sync.dma_start(out=v1b[:, :, 64:65], in_=onesv[:, :, :])

                vt_prev = None
                for nt in range(NQ):
                    sl = slice(nt * QT, (nt + 1) * QT)
                    xc, cos_c, sin_c = pref
                    if nt + 1 < NQ:
                        pref = fetch_chunk(nt + 1)
                    psq = pps.tile([P, QT], f32, tag="psq")
                    psk = pps.tile([P, QT], f32, tag="psk")
                    psv = pps.tile([P, QT], f32, tag="psv")
                    for c in range(DC):
                        st, sp = (c == 0), (c == DC - 1)
                        nc.tensor.matmul(psq, wq_sb[:, c, :], xc[:, c, :], start=st, stop=sp)
                        nc.tensor.matmul(psk, wk_sb[:, c, :], xc[:, c, :], start=st, stop=sp)
                        nc.tensor.matmul(psv, wv_sb[:, c, :], xc[:, c, :], start=st, stop=sp)
                    vt = rp.tile([P, QT], f16, tag="vt")
                    nc.scalar.copy(qT_sb[:, sl], psq)
                    nc.scalar.copy(kT_sb[:, sl], psk)
                    nc.scalar.copy(vt, psv)
                    # RoPE on this chunk (in place); sw-mul runs on GPSIMD;
                    # q swap DMAs issue from SP, k swaps from Act (spread SEQ load)
                    for src_sb, tgname, dmaeng in (
                        (qT_sb, "swq", nc.scalar),
                        (kT_sb, "swk", nc.scalar),
                    ):
                        sw = rp.tile([P, QT], f16, tag=tgname)
                        for dst0, src0 in ((0, 32), (32, 0), (64, 96), (96, 64)):
                            dmaeng.dma_start(
                                out=sw[dst0 : dst0 + 32, :],
                                in_=src_sb[src0 : src0 + 32, sl],
                            )
                        m1 = rp.tile([P, QT], f16, tag=tgname + "m")
                        nc.vector.tensor_mul(m1, src_sb[:, sl], cos_c)
                        nc.gpsimd.tensor_mul(sw, sw, sin_c)
                        nc.gpsimd.tensor_add(src_sb[:, sl], m1, sw)
                    # v~ transposes lag one chunk so PE never waits on the
                    # fresh vt copy
                    def vtrans(nt, vt):
                        for h, v1 in ((0, v1a), (1, v1b)):
                            hp = h * 64
                            pst = tpp.tile([P, DIAG, 64], f16, tag="pst", name="pst")
                            for j in range(DIAG):
                                nc.tensor.transpose(
                                    pst[:, j, :],
                                    vt[hp : hp + 64, j * KC : (j + 1) * KC],
                                    id_sb[hp : hp + 64, hp : hp + 64],
                                )
                            nc.vector.tensor_copy(
                                v1[:, DIAG * nt : DIAG * nt + DIAG, 0:64], pst
                            )
                    if vt_prev is not None:
                        vtrans(*vt_prev)
                    vt_prev = (nt, vt)
                if vt_prev is not None:
                    vtrans(*vt_prev)

            # ---- phase E: scores+exp, attnV, outproj, per q tile ----
            with (
                tc.tile_pool(name="sc_ps", bufs=2, space="PSUM") as scp,
                tc.tile_pool(name="att_ps", bufs=2, space="PSUM") as attp,
                tc.tile_pool(name="po_ps", bufs=2, space="PSUM") as pop,
                tc.tile_pool(name="asb", bufs=4) as asp,
                tc.tile_pool(name="yrow", bufs=1) as yrp,
            ):
                def score_unit(qt, kc):
                    q0 = qt * QT
                    es_all = es_od if qt % 2 else es_ev
                    if True:
                        j = kc - DIAG * qt  # >=0 on the diagonal
                        qoff = j * KC if j >= 0 else 0
                        ksl = slice(kc * KC, (kc + 1) * KC)
                        ps = scp.tile([P, 2, QT], f32, tag="sc")
                        diag = j >= 0
                        if diag:
                            # causal mask folded in as a -1e4 bias via an
                            # identity matmul into the psum (per head)
                            for h in range(2):
                                nc.tensor.matmul(
                                    ps[:, h, qoff:QT],
                                    id_sb,
                                    mask_sb[:, j, h, qoff:QT],
                                    start=True, stop=False,
                                )
                        nc.tensor.matmul(
                            ps[:, 0, qoff:QT],
                            kT_sb[0:64, ksl],
                            qT_sb[0:64, q0 + qoff : q0 + QT],
                            start=not diag, stop=True, tile_position=(0, 0),
                        )
                        nc.tensor.matmul(
                            ps[:, 1, qoff:QT],
                            kT_sb[64:128, ksl],
                            qT_sb[64:128, q0 + qoff : q0 + QT],
                            start=not diag, stop=True, tile_position=(64, 0),
                        )
                        if j >= 2:
                            # narrow per-head DVE trick exp (masked after)
                            for h in range(2):
                                nc.vector.tensor_scalar(
                                    out=es_all.bitcast(i16)[:, kc, h, qoff:QT],
                                    in0=ps[:, h, qoff:QT],
                                    scalar1=float(EXP_A),
                                    scalar2=float(EXP_B),
                                    op0=Alu.mult,
                                    op1=Alu.add,
                                )
                        elif j >= 0:
                            # narrow per-head exact exp
                            for h in range(2):
                                nc.scalar.activation(
                                    es_all[:, kc, h, qoff:QT],
                                    ps[:, h, qoff:QT],
                                    Act.Exp,
                                    scale=SCALE,
                                )
                        elif (qt, kc) in dve_exp:
                            nc.vector.tensor_scalar(
                                out=es_all.bitcast(i16)[:, kc, :, :].rearrange(
                                    "p h q -> p (h q)"
                                ),
                                in0=ps.rearrange("p h q -> p (h q)"),
                                scalar1=float(EXP_A),
                                scalar2=float(EXP_B),
                                op0=Alu.mult,
                                op1=Alu.add,
                            )
                        else:
                            nc.scalar.activation(
                                es_all[:, kc, :, :].rearrange("p h q -> p (h q)"),
                                ps.rearrange("p h q -> p (h q)"),
                                Act.Exp,
                                scale=SCALE,
                            )
                def attn_units(qt):
                    """Closures for attnV matmul steps, norms, and outproj of
                    q-tile qt; interleaved between scores chunks of qt+1 so
                    the in-order PE queue always has ready work when scores
                    stall on psum banks."""
                    es_all = es_od if qt % 2 else es_ev
                    par = qt % 2
                    q0 = qt * QT
                    state = {}
                    units = []

                    def mk_av(qcl, kc, qc):
                        def f():
                            if kc == 0:
                                state[qcl] = attp.tile([P, 2, 66], f32, tag="att", name="pa")
                            pa = state[qcl]
                            # one psum group across both heads: start=True
                            # clears has_written for the whole 2KB zero
                            # region, so only the very first matmul may start
                            nc.tensor.matmul(
                                pa[:, 0, 0:65],
                                es_all[:, kc, 0, qcl * KC : (qcl + 1) * KC],
                                v1a[:, kc, :],
                                start=(kc == 0), stop=False,
                            )
                            nc.tensor.matmul(
                                pa[:, 1, 0:65],
                                es_all[:, kc, 1, qcl * KC : (qcl + 1) * KC],
                                v1b[:, kc, :],
                                start=False, stop=(kc == qc),
                            )
                        return f

                    def mk_norm(qcl):
                        def f():
                            pa = state[qcl]
                            if debug_taps and qt == NQ - 1 and qcl == DIAG - 1:
                                pacp = asp.tile([P, 2, 66], f32, tag="pacp", name="pacp")
                                nc.vector.tensor_copy(pacp, pa)
                                nc.sync.dma_start(out=pa_out[:, :, :], in_=pacp)
                            nc.vector.reciprocal(pa[:, :, 64:65], pa[:, :, 64:65])
                            asb = asp.tile([P, P], f16, tag="asb", name="asb")
                            for h in range(2):
                                nc.vector.tensor_scalar(
                                    out=asb[:, h * 64 : (h + 1) * 64],
                                    in0=pa[:, h, 0:64],
                                    scalar1=pa[:, h, 64:65],
                                    scalar2=None,
                                    op0=Alu.mult,
                                )
                            nc.sync.dma_start_transpose(
                                out=attnT[:, par, qcl * KC : (qcl + 1) * KC],
                                in_=asb,
                            )
                            if debug_taps and qt == NQ - 1 and qcl == DIAG - 1:
                                nc.sync.dma_start(out=asb_out[:, :], in_=asb)
                        return f

                    def mk_po(oc):
                        def f():
                            if oc == 0:
                                state["yrow"] = yrp.tile([P, DC, QT], f16, tag="yrow", name="yrow")
                            po = pop.tile([P, QT], f32, tag="po", name="po")
                            nc.tensor.matmul(
                                po, wo_sb[:, oc * P : (oc + 1) * P],
                                attnT[:, par, :],
                                start=True, stop=True,
                            )
                            # tail iterations have little interleave cover:
                            # alternate the psum drain across Act/DVE there
                            if qt >= NQ - 2 and oc % 2:
                                nc.scalar.copy(state["yrow"][:, oc, :], po)
                            else:
                                nc.vector.tensor_copy(state["yrow"][:, oc, :], po)
                            if oc == DC - 1:
                                qsl = slice(q0, q0 + QT)
                                nc.scalar.dma_start(
                                    out=yT[:, qsl].rearrange("(c p) q -> p c q", p=P),
                                    in_=state["yrow"],
                                )
                        return f

                    av_units = []
                    po_units = []
                    for qcl in range(DIAG):
                        qc = DIAG * qt + qcl
                        for kc in range(qc + 1):
                            av_units.append(mk_av(qcl, kc, qc))
                        av_units.append(mk_norm(qcl))
                    for oc in range(DC):
                        po_units.append(mk_po(oc))
                    return av_units, po_units

                # software pipeline: attnV/norm of qt-1 and outproj of qt-2
                # interleave between the scores chunks of qt (proportional
                # merge) so the in-order PE queue always has ready work
                avpo = {}
                for it in range(NQ + 2):
                    units = []
                    if 1 <= it <= NQ:
                        avpo[it - 1] = attn_units(it - 1)
                        units += avpo[it - 1][0]
                    if it >= 2:
                        units += avpo.pop(it - 2)[1]
                    if it < NQ:
                        nkc = DIAG * it + DIAG
                        done = 0
                        for kc in range(nkc):
                            score_unit(it, kc)
                            want = (kc + 1) * len(units) // nkc
                            while done < want:
                                units[done]()
                                done += 1
                    else:
                        # tail: interleave attnV and outproj streams
                        a, b = (avpo[it - 1][0], units[len(avpo[it - 1][0]):]) if (
                            it - 1
                        ) in avpo else (units, [])
                        ia = ib = 0
                        n = max(len(a), 1)
                        for i_ in range(len(a)):
                            a[i_]()
                            want = (i_ + 1) * len(b) // n
                            while ib < want:
                                b[ib]()
                                ib += 1
                        while ib < len(b):
                            b[ib]()
                            ib += 1
                if debug_taps:
                    nc.sync.dma_start(out=qT_out[:, :], in_=qT_sb)
                    nc.sync.dma_start(out=kT_out[:, :], in_=kT_sb)
                    nc.sync.dma_start(out=v1a_out[:, :, :], in_=v1a)
                    nc.sync.dma_start(out=es_out[:, :, :, :], in_=es_od)
                    nc.sync.dma_start(out=at_out[:, :, :], in_=attnT)

    nc.compile()
    _BUILD_CACHE[key] = nc
    return nc


def host_prep(x, Wq, Wk, Wv, Wo, S=S_FULL):
    """Build per-core input maps (numpy, fp16)."""
    x = np.asarray(x, np.float32).reshape(S, D)
    xT = np.ascontiguousarray(x.T, dtype=np.float16)

    perm64 = np.concatenate([np.arange(0, 64, 2), np.arange(1, 64, 2)])
    j32 = np.arange(32, dtype=np.float64)
    rates = THETA ** (-2.0 * j32 / DK)
    pos = np.arange(S, dtype=np.float64)
    ang = rates[:, None] * pos[None, :]  # (32, S)
    cos32 = np.cos(ang)
    sin32 = np.sin(ang)
    cosd = np.tile(cos32, (4, 1)).astype(np.float16)  # (128, S)
    signs = np.repeat([-1.0, 1.0, -1.0, 1.0], 32)[:, None]
    sind = (np.tile(sin32, (4, 1)) * signs).astype(np.float16)

    DIAG = QT // KC
    r = np.arange(P)[:, None, None, None]
    jj = np.arange(DIAG)[None, :, None, None]
    q_local = np.arange(QT)[None, None, None, :]
    maskd = np.broadcast_to(
        np.where(q_local >= jj * KC + r, 0.0, -10000.0), (P, DIAG, 2, QT)
    ).astype(np.float16)

    ident = np.eye(P, dtype=np.float16)

    in_maps = []
    for g in range(N_CORES):
        h0, h1 = 2 * g, 2 * g + 1
        idx_qk = np.concatenate([h0 * DK + perm64, h1 * DK + perm64])
        idx_v = np.arange(h0 * DK, h0 * DK + 2 * DK)
        in_maps.append(
            {
                "xT": xT,
                "wqT": np.ascontiguousarray(np.asarray(Wq)[idx_qk, :].T, dtype=np.float16),
                "wkT": np.ascontiguousarray(np.asarray(Wk)[idx_qk, :].T, dtype=np.float16),
                "wvT": np.ascontiguousarray(np.asarray(Wv)[idx_v, :].T, dtype=np.float16),
                "woT": np.ascontiguousarray(np.asarray(Wo)[:, idx_v].T, dtype=np.float16),
                "cosd": cosd,
                "sind": sind,
                "maskd": maskd,
                "ident": ident,
                "onesv": np.ones((P, S // KC, 1), np.float16),
            }
        )
    return in_maps


def run_cores(x, Wq, Wk, Wv, Wo, S=S_FULL, core_ids=None, trace=False):
    from concourse.bass_utils import run_bass_kernel_spmd

    nc = build(S)
    in_maps = host_prep(x, Wq, Wk, Wv, Wo, S=S)
    if core_ids is None:
        core_ids = list(range(N_CORES))
    in_maps = in_maps[: len(core_ids)]
    res = run_bass_kernel_spmd(nc, in_maps, core_ids, trace=trace)
    return res


def kernel(x, Wq, Wk, Wv, Wo):
    x = np.asarray(x, np.float32)
    res = run_cores(x, np.asarray(Wq), np.asarray(Wk), np.asarray(Wv), np.asarray(Wo))
    y = np.zeros((D, S_FULL), np.float64)
    for r in res.results:
        y += r["yT"].astype(np.float64)
    return np.ascontiguousarray(y.T, dtype=np.float32).reshape(1, S_FULL, D)
